# revision 6
# baseline (speedup 1.0000x reference)
"""Trainium2 Bass kernel for nn_DIoUAnswerSpanLoss.

Algorithm
---------
The reference builds a [B, L, L] score matrix score[b,i,j] = ls[b,i] + le[b,j]
(log-softmaxed logits), masks the lower triangle and pad positions, takes
argmax over (i, j), then computes a DIoU-style loss from the resulting integer
span positions (and the ground-truth spans).

Two exact reductions make this cheap:
  1. log_softmax subtracts a per-row constant, which never changes any
     argmax, so raw logits can be used directly for position finding.
  2. max_i<=j (msl[i] + el[j]) = cummax(msl)[j] + el[j], so the [L, L]
     matrix never needs materializing - a prefix max suffices.

Per row: msl = mask ? sl : -BIG; m = cummax(msl); col = mask ? m + el : -BIG;
j* = first argmax(col); v* = m[j*]; i* = first index with m == v*.

Distribution: the whole computation per core is ~10us of straight-line work,
while an 8-core AllReduce of even 16 bytes costs ~45us of latency on this
fabric. So instead of sharding the batch and reducing, every core receives
the full input and computes the full scalar loss independently - zero
communication, no kernel-entry collective barrier, and the answer is read
from core 0.

Layout: each of the 32 rows occupies 4 SBUF partitions x 512 elements
(p = 4*row + seg, j = 512*seg + f). The prefix max runs as a segmented
tensor_tensor_scan; segment boundaries are stitched with a second scan over
the PE-transposed segment ends (a -1e30 additive "reset" at each row start
keeps rows independent). Cross-partition moves use fp32 matmuls with
one-hot/identity constants (exact: every product is x*1.0 or x*0.0).

First-occurrence argmax semantics (matching jnp.argmax) use
(value == max) * (iota - 4096) followed by reduce_min; the -4096 offset makes
candidates negative so zeros from non-matching lanes never win, and it
cancels in every DIoU term (only differences and pairwise min/max of
positions appear), so ground-truth positions are passed pre-offset.
"""

import numpy as np

import concourse.bass as bass
import concourse.bacc as bacc
import concourse.mybir as mybir
from concourse import tile
from concourse.bass_utils import run_bass_kernel_spmd

B, L = 32, 2048
NCORES = 8
SEG = 4                    # segments (partitions) per row
FREE = L // SEG            # 512 elements per segment
P = B * SEG                # 128 partitions
NEGBIG = -1.0e30
OFF = 4096.0               # index offset (positions stored as idx - OFF)

F32 = mybir.dt.float32
A = mybir.AluOpType
AX = mybir.AxisListType.X

_CACHE = {}


def _tt(nc, out, a, b, op):
    # plain tensor_tensor; Bacc's generate_event_semaphores legalizes any
    # multi-wait instructions, and TT runs ~1.5x faster than the STT form
    nc.vector.tensor_tensor(out, a, b, op)


def _b3(ap, inner):
    """View [1, N] as [1, N, inner] with a 0-step (broadcast) inner dim."""
    return bass.AP(ap.tensor, ap.offset, list(ap.ap) + [[0, inner]])


def _v3(ap, inner):
    """View [1, N*inner] as [1, N, inner]."""
    return ap.rearrange("p (a b) -> p a b", b=inner)


def _build_program():
    nc = bacc.Bacc("TRN2", target_bir_lowering=False, debug=False, num_devices=NCORES)

    sl = nc.dram_tensor("sl", [P, FREE], F32, kind="ExternalInput")
    el = nc.dram_tensor("el", [P, FREE], F32, kind="ExternalInput")
    cid = nc.dram_tensor("cid", [P, FREE], F32, kind="ExternalInput")
    gt2 = nc.dram_tensor("gt2", [1, 2 * B], F32, kind="ExternalInput")
    ones1 = nc.dram_tensor("ones1", [1, 1], F32, kind="ExternalInput")
    emat = nc.dram_tensor("emat", [B, P], F32, kind="ExternalInput")
    resetv = nc.dram_tensor("resetv", [1, P], F32, kind="ExternalInput")
    rbase2 = nc.dram_tensor("rbase2", [1, 2 * B], F32, kind="ExternalInput")
    loss = nc.dram_tensor("loss", [1], F32, kind="ExternalOutput")

    with tile.TileContext(nc) as tc:
        with (
            tc.tile_pool(name="sb", bufs=1) as sb,
            tc.tile_pool(name="ps", bufs=4, space="PSUM") as ps,
        ):
            # ---- loads (critical ones first) ----
            sl_s = sb.tile([P, FREE], F32)
            cid_s = sb.tile([P, FREE], F32)
            el_s = sb.tile([P, FREE], F32)
            resetv_s = sb.tile([1, P], F32)
            ones1_s = sb.tile([1, 1], F32)
            emat_s = sb.tile([B, P], F32)
            gt2_s = sb.tile([1, 2 * B], F32)
            rbase2_s = sb.tile([1, 2 * B], F32)
            nc.sync.dma_start(sl_s[:], sl[:])
            nc.sync.dma_start(cid_s[:], cid[:])
            nc.sync.dma_start(el_s[:], el[:])
            nc.sync.dma_start(resetv_s[:], resetv[:])
            nc.sync.dma_start(ones1_s[:], ones1[:])
            nc.sync.dma_start(emat_s[:], emat[:])
            nc.sync.dma_start(gt2_s[:], gt2[:])
            nc.sync.dma_start(rbase2_s[:], rbase2[:])

            # ---- constants generated on-device (GpSimd, off critical path) ----
            # identity: (f - p) == 0
            idi = sb.tile([P, P], mybir.dt.int32)
            nc.gpsimd.iota(idi[:], pattern=[[1, P]], base=0, channel_multiplier=-1)
            id128_s = sb.tile([P, P], F32)
            nc.gpsimd.tensor_scalar(id128_s[:], idi[:], 0, None, A.is_equal)
            # global index: 512*p + f - 65536 (all negative, exact in fp32)
            ioi = sb.tile([P, FREE], mybir.dt.int32)
            nc.gpsimd.iota(
                ioi[:], pattern=[[1, FREE]], base=-(P * FREE), channel_multiplier=FREE
            )
            iotag_s = sb.tile([P, FREE], F32)
            nc.gpsimd.tensor_copy(iotag_s[:], ioi[:])

            # ---- masked start logits ----
            penalty = sb.tile([P, FREE], F32)
            nc.vector.tensor_scalar(
                penalty[:], cid_s[:], 0.0, NEGBIG, A.is_equal, A.mult
            )
            msl = sb.tile([P, FREE], F32)
            _tt(nc, msl[:], sl_s[:], penalty[:], A.add)

            # ---- segmented prefix max (within each 512-elem segment) ----
            mseg = sb.tile([P, FREE], F32)
            nc.vector.tensor_tensor_scan(
                mseg[:], msl[:], msl[:], NEGBIG, A.max, A.max
            )

            # ---- stitch segments: exclusive cross-segment prefix max ----
            psA = ps.tile([1, P], F32, tag="ps")
            nc.tensor.matmul(psA[:], mseg[:, FREE - 1 : FREE], id128_s[:])
            inclT = sb.tile([1, P], F32)
            nc.vector.tensor_tensor_scan(
                inclT[:], resetv_s[:], psA[:], NEGBIG, A.add, A.max
            )
            exclT = sb.tile([1, P], F32)
            nc.vector.memset(exclT[0:1, 0:1], NEGBIG)
            _tt(nc, exclT[0:1, 1:P], inclT[0:1, 0 : P - 1], resetv_s[0:1, 1:P], A.add)
            psE = ps.tile([P, 1], F32, tag="ps")
            nc.tensor.matmul(psE[:], exclT[:], ones1_s[:])
            m = sb.tile([P, FREE], F32)
            nc.vector.tensor_scalar(m[:], mseg[:], psE[:], None, A.max)

            # ---- column scores: col = m + el + penalty ----
            mel = sb.tile([P, FREE], F32)
            _tt(nc, mel[:], el_s[:], penalty[:], A.add)
            col = sb.tile([P, FREE], F32)
            _tt(nc, col[:], m[:], mel[:], A.add)

            # ---- per-partition max / first-argmax / gathered m value ----
            pmt = sb.tile([P, 1], F32)
            pm = pmt[:]
            nc.vector.reduce_max(pm, col[:], axis=AX)
            candj = sb.tile([P, FREE], F32)
            nc.vector.scalar_tensor_tensor(
                candj[:], col[:], pm, iotag_s[:], A.is_equal, A.mult
            )
            pjt = sb.tile([P, 1], F32)
            pj = pjt[:]
            nc.vector.tensor_reduce(pj, candj[:], axis=AX, op=A.min)
            candv = sb.tile([P, FREE], F32)
            nc.vector.scalar_tensor_tensor(
                candv[:], iotag_s[:], pj, m[:], A.is_equal, A.mult
            )
            pvt = sb.tile([P, 1], F32)
            pv = pvt[:]
            nc.vector.reduce_sum(pv, candv[:], axis=AX)

            # ---- transpose pm/pj/pv into one [1, 3P] row (partition 0) ----
            psW = ps.tile([1, 3 * P], F32, tag="ps")
            nc.tensor.matmul(psW[0:1, 0:P], pm, id128_s[:])
            nc.tensor.matmul(psW[0:1, P : 2 * P], pj, id128_s[:])
            nc.tensor.matmul(psW[0:1, 2 * P : 3 * P], pv, id128_s[:])
            sT = sb.tile([1, 3 * P], F32)
            nc.vector.tensor_copy(sT[:], psW[:])
            pos = sb.tile([1, 2 * B], F32)   # i*-4096 || j*-4096
            i4 = pos[0:1, 0:B]
            j4 = pos[0:1, B : 2 * B]
            m4 = sb.tile([1, B], F32)
            nc.vector.reduce_max(m4[:], _v3(sT[0:1, 0:P], SEG), axis=AX)
            c1 = sb.tile([1, P], F32)
            _tt(nc, _v3(c1[:], SEG), _v3(sT[0:1, 0:P], SEG), _b3(m4[:], SEG), A.is_equal)
            candjT = sb.tile([1, P], F32)
            _tt(nc, candjT[:], c1[:], sT[0:1, P : 2 * P], A.mult)
            nc.vector.tensor_reduce(j4, _v3(candjT[:], SEG), axis=AX, op=A.min)
            c2 = sb.tile([1, P], F32)
            _tt(nc, _v3(c2[:], SEG), _v3(candjT[:], SEG), _b3(j4, SEG), A.is_equal)
            candvT = sb.tile([1, P], F32)
            _tt(nc, candvT[:], c2[:], sT[0:1, 2 * P : 3 * P], A.mult)
            v4 = sb.tile([1, B], F32)
            nc.vector.reduce_sum(v4[:], _v3(candvT[:], SEG), axis=AX)

            # ---- broadcast v* back per partition, find i* ----
            psV = ps.tile([B, 1], F32, tag="ps")
            nc.tensor.matmul(psV[:], v4[:], ones1_s[:])
            v4t = sb.tile([B, 1], F32)
            nc.vector.tensor_copy(v4t[:], psV[:])
            psVb = ps.tile([P, 1], F32, tag="ps")
            nc.tensor.matmul(psVb[:], emat_s[:], v4t[:])
            candi = sb.tile([P, FREE], F32)
            nc.vector.scalar_tensor_tensor(
                candi[:], m[:], psVb[:], iotag_s[:], A.is_equal, A.mult
            )
            pi = sb.tile([P, 1], F32)
            nc.vector.tensor_reduce(pi[:], candi[:], axis=AX, op=A.min)
            psI = ps.tile([1, P], F32, tag="ps")
            nc.tensor.matmul(psI[:], pi[:], id128_s[:])
            nc.vector.tensor_reduce(i4, _v3(psI[0:1, :], SEG), axis=AX, op=A.min)

            # ---- DIoU from positions (all offsets cancel) ----
            # global-index positions -> per-row (idx - 4096) form
            posn = sb.tile([1, 2 * B], F32)
            _tt(nc, posn[:], pos[:], rbase2_s[:], A.subtract)
            i4n = posn[0:1, 0:B]
            j4n = posn[0:1, B : 2 * B]
            te2 = sb.tile([1, 2 * B], F32)   # t_es || t_gs
            _tt(nc, te2[0:1, 0:B], j4n, i4n, A.subtract)
            _tt(nc, te2[0:1, B : 2 * B], gt2_s[0:1, B : 2 * B], gt2_s[0:1, 0:B], A.subtract)
            cdg = sb.tile([1, 2 * B], F32)   # cd || gcd = (x+1)*0.5
            nc.vector.tensor_scalar(cdg[:], te2[:], 1.0, 0.5, A.add, A.mult)
            mnx = sb.tile([1, 2 * B], F32)   # min(i4,gsp) || min(j4,gep)
            _tt(nc, mnx[:], posn[:], gt2_s[:], A.min)
            mxx = sb.tile([1, 2 * B], F32)   # max(i4,gsp) || max(j4,gep)
            _tt(nc, mxx[:], posn[:], gt2_s[:], A.max)
            sq = sb.tile([1, 3 * B], F32)    # d1^2 || d2^2 || tsum
            d1 = sb.tile([1, B], F32)
            _tt(nc, d1[:], cdg[0:1, 0:B], cdg[0:1, B : 2 * B], A.subtract)
            _tt(nc, sq[0:1, 0:B], d1[:], d1[:], A.mult)
            d2 = sb.tile([1, B], F32)
            _tt(nc, d2[:], mxx[0:1, B : 2 * B], mnx[0:1, 0:B], A.subtract)
            _tt(nc, sq[0:1, B : 2 * B], d2[:], d2[:], A.mult)
            _tt(nc, sq[0:1, 2 * B : 3 * B], te2[0:1, 0:B], te2[0:1, B : 2 * B], A.add)
            s123 = sb.tile([1, 3], F32)      # [s1, s2, s3]
            nc.vector.reduce_sum(s123[:], _v3(sq[:], B), axis=AX)
            dI = sb.tile([1, 1], F32)        # I = min(ep,gep)[0] - max(sp,gsp)[0]
            _tt(nc, dI[:], mnx[0:1, B : B + 1], mxx[0:1, 0:1], A.subtract)

            # loss = 1 - I/(s3 - 32*I) + s1/s2
            u = sb.tile([1, 1], F32)
            nc.vector.scalar_tensor_tensor(
                u[:], dI[:], -float(B), s123[0:1, 2:3], A.mult, A.add
            )
            ru = sb.tile([1, 1], F32)
            nc.vector.reciprocal(ru[:], u[:])
            iou = sb.tile([1, 1], F32)
            _tt(nc, iou[:], dI[:], ru[:], A.mult)
            r2 = sb.tile([1, 1], F32)
            nc.vector.reciprocal(r2[:], s123[0:1, 1:2])
            cl = sb.tile([1, 1], F32)
            _tt(nc, cl[:], s123[0:1, 0:1], r2[:], A.mult)
            tmp2 = sb.tile([1, 1], F32)
            nc.vector.scalar_tensor_tensor(
                tmp2[:], iou[:], -1.0, cl[:], A.mult, A.add
            )
            lossv = sb.tile([1, 1], F32)
            nc.vector.tensor_scalar(lossv[:], tmp2[:], 1.0, None, A.add)
            nc.sync.dma_start(loss[:], lossv[:])

    nc.compile()
    return nc


def _constants():
    ones1 = np.ones((1, 1), dtype=np.float32)
    emat = np.zeros((B, P), dtype=np.float32)
    for k in range(B):
        emat[k, k * SEG : (k + 1) * SEG] = 1.0
    resetv = np.zeros((1, P), dtype=np.float32)
    resetv[0, ::SEG] = NEGBIG
    # global index of row start, in (idx - 65536) space, minus the -4096 form:
    # posn = pos_global - rbase2 with rbase2[k] = 2048*(k % 32) - 61440
    rbase2 = (
        2048.0 * (np.arange(2 * B, dtype=np.float32) % B) - (P * FREE - OFF)
    ).reshape(1, 2 * B).astype(np.float32)
    return {
        "ones1": ones1,
        "emat": emat,
        "resetv": resetv,
        "rbase2": rbase2,
    }


def build_in_maps(c_ids, gt_start_positions, gt_end_positions, start_logits, end_logits):
    consts = _constants()
    cidf = np.ascontiguousarray(np.asarray(c_ids), dtype=np.float32).reshape(P, FREE)
    slf = np.ascontiguousarray(np.asarray(start_logits), dtype=np.float32).reshape(P, FREE)
    elf = np.ascontiguousarray(np.asarray(end_logits), dtype=np.float32).reshape(P, FREE)
    gt2 = np.concatenate(
        [
            np.asarray(gt_start_positions).astype(np.float32) - np.float32(OFF),
            np.asarray(gt_end_positions).astype(np.float32) - np.float32(OFF),
        ]
    ).reshape(1, 2 * B)
    core_map = {"sl": slf, "el": elf, "cid": cidf, "gt2": gt2, **consts}
    return [dict(core_map) for _ in range(NCORES)]


def kernel(c_ids, gt_start_positions, gt_end_positions, start_logits, end_logits):
    if "nc" not in _CACHE:
        _CACHE["nc"] = _build_program()
    nc = _CACHE["nc"]
    in_maps = build_in_maps(
        c_ids, gt_start_positions, gt_end_positions, start_logits, end_logits
    )
    res = run_bass_kernel_spmd(nc, in_maps, core_ids=list(range(NCORES)))
    return np.asarray(res.results[0]["loss"], dtype=np.float32).reshape(())


# revision 8
# speedup vs baseline: 1.0530x; 1.0530x over previous
"""Trainium2 Bass kernel for nn_DIoUAnswerSpanLoss.

Algorithm
---------
The reference builds a [B, L, L] score matrix score[b,i,j] = ls[b,i] + le[b,j]
(log-softmaxed logits), masks the lower triangle and pad positions, takes
argmax over (i, j), then computes a DIoU-style loss from the resulting integer
span positions (and the ground-truth spans).

Two exact reductions make this cheap:
  1. log_softmax subtracts a per-row constant, which never changes any
     argmax, so raw logits can be used directly for position finding.
  2. max_i<=j (msl[i] + el[j]) = cummax(msl)[j] + el[j], so the [L, L]
     matrix never needs materializing - a prefix max suffices.

Per row: msl = mask ? sl : -BIG; m = cummax(msl); col = mask ? m + el : -BIG;
j* = first argmax(col); v* = m[j*]; i* = first index with m == v*.

Distribution: the whole computation per core is ~10us of straight-line work,
while an 8-core AllReduce of even 16 bytes costs ~45us of latency on this
fabric. So instead of sharding the batch and reducing, every core receives
the full input and computes the full scalar loss independently - zero
communication, no kernel-entry collective barrier, and the answer is read
from core 0.

Layout: each of the 32 rows occupies 4 SBUF partitions x 512 elements
(p = 4*row + seg, j = 512*seg + f). The prefix max runs as a segmented
tensor_tensor_scan; segment boundaries are stitched with a second scan over
the PE-transposed segment ends (a -1e30 additive "reset" at each row start
keeps rows independent). Cross-partition moves use fp32 matmuls with
one-hot/identity constants (exact: every product is x*1.0 or x*0.0).

First-occurrence argmax semantics (matching jnp.argmax) use
(value == max) * (iota - 4096) followed by reduce_min; the -4096 offset makes
candidates negative so zeros from non-matching lanes never win, and it
cancels in every DIoU term (only differences and pairwise min/max of
positions appear), so ground-truth positions are passed pre-offset.
"""

import numpy as np

import concourse.bass as bass
import concourse.bacc as bacc
import concourse.mybir as mybir
from concourse import tile
from concourse.bass_utils import run_bass_kernel_spmd

B, L = 32, 2048
NCORES = 8
SEG = 4                    # segments (partitions) per row
FREE = L // SEG            # 512 elements per segment
P = B * SEG                # 128 partitions
NEGBIG = -1.0e30
OFF = 4096.0               # index offset (positions stored as idx - OFF)

F32 = mybir.dt.float32
A = mybir.AluOpType
AX = mybir.AxisListType.X

_CACHE = {}


def _tt(nc, out, a, b, op):
    # plain tensor_tensor; Bacc's generate_event_semaphores legalizes any
    # multi-wait instructions, and TT runs ~1.5x faster than the STT form
    nc.vector.tensor_tensor(out, a, b, op)


def _b3(ap, inner):
    """View [1, N] as [1, N, inner] with a 0-step (broadcast) inner dim."""
    return bass.AP(ap.tensor, ap.offset, list(ap.ap) + [[0, inner]])


def _v3(ap, inner):
    """View [1, N*inner] as [1, N, inner]."""
    return ap.rearrange("p (a b) -> p a b", b=inner)


def _build_program():
    nc = bacc.Bacc("TRN2", target_bir_lowering=False, debug=False, num_devices=NCORES)

    sl = nc.dram_tensor("sl", [P, FREE], F32, kind="ExternalInput")
    el = nc.dram_tensor("el", [P, FREE], F32, kind="ExternalInput")
    cid = nc.dram_tensor("cid", [P, FREE], F32, kind="ExternalInput")
    gt2 = nc.dram_tensor("gt2", [1, 2 * B], F32, kind="ExternalInput")
    ones1 = nc.dram_tensor("ones1", [1, 1], F32, kind="ExternalInput")
    emat = nc.dram_tensor("emat", [B, P], F32, kind="ExternalInput")
    resetv = nc.dram_tensor("resetv", [1, P], F32, kind="ExternalInput")
    rbase2 = nc.dram_tensor("rbase2", [1, 2 * B], F32, kind="ExternalInput")
    id128 = nc.dram_tensor("id128", [P, P], F32, kind="ExternalInput")
    iotag = nc.dram_tensor("iotag", [P, FREE], F32, kind="ExternalInput")
    loss = nc.dram_tensor("loss", [1], F32, kind="ExternalOutput")

    with tile.TileContext(nc) as tc:
        with (
            tc.tile_pool(name="sb", bufs=1) as sb,
            tc.tile_pool(name="ps", bufs=4, space="PSUM") as ps,
        ):
            # ---- loads (critical ones first) ----
            sl_s = sb.tile([P, FREE], F32)
            cid_s = sb.tile([P, FREE], F32)
            el_s = sb.tile([P, FREE], F32)
            resetv_s = sb.tile([1, P], F32)
            ones1_s = sb.tile([1, 1], F32)
            emat_s = sb.tile([B, P], F32)
            gt2_s = sb.tile([1, 2 * B], F32)
            rbase2_s = sb.tile([1, 2 * B], F32)
            nc.sync.dma_start(sl_s[:], sl[:])
            nc.sync.dma_start(cid_s[:], cid[:])
            nc.sync.dma_start(el_s[:], el[:])
            nc.sync.dma_start(resetv_s[:], resetv[:])
            nc.sync.dma_start(ones1_s[:], ones1[:])
            nc.sync.dma_start(emat_s[:], emat[:])
            nc.sync.dma_start(gt2_s[:], gt2[:])
            nc.sync.dma_start(rbase2_s[:], rbase2[:])
            id128_s = sb.tile([P, P], F32)
            nc.sync.dma_start(id128_s[:], id128[:])
            iotag_s = sb.tile([P, FREE], F32)
            nc.sync.dma_start(iotag_s[:], iotag[:])

            # ---- masked start logits ----
            penalty = sb.tile([P, FREE], F32)
            nc.vector.tensor_scalar(
                penalty[:], cid_s[:], 0.0, NEGBIG, A.is_equal, A.mult
            )
            msl = sb.tile([P, FREE], F32)
            _tt(nc, msl[:], sl_s[:], penalty[:], A.add)

            # ---- segmented prefix max (within each 512-elem segment) ----
            mseg = sb.tile([P, FREE], F32)
            nc.vector.tensor_tensor_scan(
                mseg[:], msl[:], msl[:], NEGBIG, A.max, A.max
            )

            # ---- stitch segments: exclusive cross-segment prefix max ----
            psA = ps.tile([1, P], F32, tag="ps")
            nc.tensor.matmul(psA[:], mseg[:, FREE - 1 : FREE], id128_s[:])
            inclT = sb.tile([1, P], F32)
            nc.vector.tensor_tensor_scan(
                inclT[:], resetv_s[:], psA[:], NEGBIG, A.add, A.max
            )
            exclT = sb.tile([1, P], F32)
            nc.vector.memset(exclT[0:1, 0:1], NEGBIG)
            _tt(nc, exclT[0:1, 1:P], inclT[0:1, 0 : P - 1], resetv_s[0:1, 1:P], A.add)
            psE = ps.tile([P, 1], F32, tag="ps")
            nc.tensor.matmul(psE[:], exclT[:], ones1_s[:])
            m = sb.tile([P, FREE], F32)
            nc.vector.tensor_scalar(m[:], mseg[:], psE[:], None, A.max)

            # ---- column scores: col = m + el + penalty ----
            mel = sb.tile([P, FREE], F32)
            _tt(nc, mel[:], el_s[:], penalty[:], A.add)
            col = sb.tile([P, FREE], F32)
            _tt(nc, col[:], m[:], mel[:], A.add)

            # ---- per-partition max / first-argmax / gathered m value ----
            top8 = sb.tile([P, 8], F32)
            nc.vector.max(top8[:], col[:])
            pm = top8[:, 0:1]
            idx8 = sb.tile([P, 8], mybir.dt.uint32)
            nc.vector.max_index(idx8[:], top8[:], col[:])
            idxf = sb.tile([P, 1], F32)
            nc.vector.tensor_copy(idxf[:], idx8[:, 0:1])
            pjt = sb.tile([P, 1], F32)
            pj = pjt[:]
            nc.vector.tensor_scalar(pj, idxf[:], iotag_s[:, 0:1], None, A.add)
            candv = sb.tile([P, FREE], F32)
            nc.vector.scalar_tensor_tensor(
                candv[:], iotag_s[:], pj, m[:], A.is_equal, A.mult
            )
            pvt = sb.tile([P, 1], F32)
            pv = pvt[:]
            nc.vector.reduce_sum(pv, candv[:], axis=AX)

            # ---- transpose pm/pj/pv into one [1, 3P] row (partition 0) ----
            psW = ps.tile([1, 3 * P], F32, tag="ps")
            nc.tensor.matmul(psW[0:1, 0:P], pm, id128_s[:])
            nc.tensor.matmul(psW[0:1, P : 2 * P], pj, id128_s[:])
            nc.tensor.matmul(psW[0:1, 2 * P : 3 * P], pv, id128_s[:])
            sT = sb.tile([1, 3 * P], F32)
            nc.vector.tensor_copy(sT[:], psW[:])
            pos = sb.tile([1, 2 * B], F32)   # i*-4096 || j*-4096
            i4 = pos[0:1, 0:B]
            j4 = pos[0:1, B : 2 * B]
            m4 = sb.tile([1, B], F32)
            nc.vector.reduce_max(m4[:], _v3(sT[0:1, 0:P], SEG), axis=AX)
            c1 = sb.tile([1, P], F32)
            _tt(nc, _v3(c1[:], SEG), _v3(sT[0:1, 0:P], SEG), _b3(m4[:], SEG), A.is_equal)
            candjT = sb.tile([1, P], F32)
            _tt(nc, candjT[:], c1[:], sT[0:1, P : 2 * P], A.mult)
            nc.vector.tensor_reduce(j4, _v3(candjT[:], SEG), axis=AX, op=A.min)
            c2 = sb.tile([1, P], F32)
            _tt(nc, _v3(c2[:], SEG), _v3(candjT[:], SEG), _b3(j4, SEG), A.is_equal)
            candvT = sb.tile([1, P], F32)
            _tt(nc, candvT[:], c2[:], sT[0:1, 2 * P : 3 * P], A.mult)
            v4 = sb.tile([1, B], F32)
            nc.vector.reduce_sum(v4[:], _v3(candvT[:], SEG), axis=AX)

            # ---- broadcast v* back per partition, find i* ----
            psV = ps.tile([B, 1], F32, tag="ps")
            nc.tensor.matmul(psV[:], v4[:], ones1_s[:])
            v4t = sb.tile([B, 1], F32)
            nc.vector.tensor_copy(v4t[:], psV[:])
            psVb = ps.tile([P, 1], F32, tag="ps")
            nc.tensor.matmul(psVb[:], emat_s[:], v4t[:])
            candi = sb.tile([P, FREE], F32)
            nc.vector.scalar_tensor_tensor(
                candi[:], m[:], psVb[:], iotag_s[:], A.is_equal, A.mult
            )
            pi = sb.tile([P, 1], F32)
            nc.vector.tensor_reduce(pi[:], candi[:], axis=AX, op=A.min)
            psI = ps.tile([1, P], F32, tag="ps")
            nc.tensor.matmul(psI[:], pi[:], id128_s[:])
            nc.vector.tensor_reduce(i4, _v3(psI[0:1, :], SEG), axis=AX, op=A.min)

            # ---- DIoU from positions (all offsets cancel) ----
            # global-index positions -> per-row (idx - 4096) form
            posn = sb.tile([1, 2 * B], F32)
            _tt(nc, posn[:], pos[:], rbase2_s[:], A.subtract)
            i4n = posn[0:1, 0:B]
            j4n = posn[0:1, B : 2 * B]
            te2 = sb.tile([1, 2 * B], F32)   # t_es || t_gs
            _tt(nc, te2[0:1, 0:B], j4n, i4n, A.subtract)
            _tt(nc, te2[0:1, B : 2 * B], gt2_s[0:1, B : 2 * B], gt2_s[0:1, 0:B], A.subtract)
            cdg = sb.tile([1, 2 * B], F32)   # cd || gcd = (x+1)*0.5
            nc.vector.tensor_scalar(cdg[:], te2[:], 1.0, 0.5, A.add, A.mult)
            mnx = sb.tile([1, 2 * B], F32)   # min(i4,gsp) || min(j4,gep)
            _tt(nc, mnx[:], posn[:], gt2_s[:], A.min)
            mxx = sb.tile([1, 2 * B], F32)   # max(i4,gsp) || max(j4,gep)
            _tt(nc, mxx[:], posn[:], gt2_s[:], A.max)
            sq = sb.tile([1, 3 * B], F32)    # d1^2 || d2^2 || tsum
            d1 = sb.tile([1, B], F32)
            _tt(nc, d1[:], cdg[0:1, 0:B], cdg[0:1, B : 2 * B], A.subtract)
            _tt(nc, sq[0:1, 0:B], d1[:], d1[:], A.mult)
            d2 = sb.tile([1, B], F32)
            _tt(nc, d2[:], mxx[0:1, B : 2 * B], mnx[0:1, 0:B], A.subtract)
            _tt(nc, sq[0:1, B : 2 * B], d2[:], d2[:], A.mult)
            _tt(nc, sq[0:1, 2 * B : 3 * B], te2[0:1, 0:B], te2[0:1, B : 2 * B], A.add)
            s123 = sb.tile([1, 3], F32)      # [s1, s2, s3]
            nc.vector.reduce_sum(s123[:], _v3(sq[:], B), axis=AX)
            dI = sb.tile([1, 1], F32)        # I = min(ep,gep)[0] - max(sp,gsp)[0]
            _tt(nc, dI[:], mnx[0:1, B : B + 1], mxx[0:1, 0:1], A.subtract)

            # loss = 1 - I/(s3 - 32*I) + s1/s2
            u = sb.tile([1, 1], F32)
            nc.vector.scalar_tensor_tensor(
                u[:], dI[:], -float(B), s123[0:1, 2:3], A.mult, A.add
            )
            ru = sb.tile([1, 1], F32)
            nc.vector.reciprocal(ru[:], u[:])
            iou = sb.tile([1, 1], F32)
            _tt(nc, iou[:], dI[:], ru[:], A.mult)
            r2 = sb.tile([1, 1], F32)
            nc.vector.reciprocal(r2[:], s123[0:1, 1:2])
            cl = sb.tile([1, 1], F32)
            _tt(nc, cl[:], s123[0:1, 0:1], r2[:], A.mult)
            tmp2 = sb.tile([1, 1], F32)
            nc.vector.scalar_tensor_tensor(
                tmp2[:], iou[:], -1.0, cl[:], A.mult, A.add
            )
            lossv = sb.tile([1, 1], F32)
            nc.vector.tensor_scalar(lossv[:], tmp2[:], 1.0, None, A.add)
            nc.sync.dma_start(loss[:], lossv[:])

    nc.compile()
    return nc


def _constants():
    ones1 = np.ones((1, 1), dtype=np.float32)
    emat = np.zeros((B, P), dtype=np.float32)
    for k in range(B):
        emat[k, k * SEG : (k + 1) * SEG] = 1.0
    resetv = np.zeros((1, P), dtype=np.float32)
    resetv[0, ::SEG] = NEGBIG
    # global index of row start, in (idx - 65536) space, minus the -4096 form:
    # posn = pos_global - rbase2 with rbase2[k] = 2048*(k % 32) - 61440
    rbase2 = (
        2048.0 * (np.arange(2 * B, dtype=np.float32) % B) - (P * FREE - OFF)
    ).reshape(1, 2 * B).astype(np.float32)
    id128 = np.eye(P, dtype=np.float32)
    iotag = (
        512.0 * np.arange(P, dtype=np.float32)[:, None]
        + np.arange(FREE, dtype=np.float32)[None, :]
        - float(P * FREE)
    ).astype(np.float32)
    return {
        "ones1": ones1,
        "emat": emat,
        "resetv": resetv,
        "rbase2": rbase2,
        "id128": id128,
        "iotag": iotag,
    }


def build_in_maps(c_ids, gt_start_positions, gt_end_positions, start_logits, end_logits):
    consts = _constants()
    cidf = np.ascontiguousarray(np.asarray(c_ids), dtype=np.float32).reshape(P, FREE)
    slf = np.ascontiguousarray(np.asarray(start_logits), dtype=np.float32).reshape(P, FREE)
    elf = np.ascontiguousarray(np.asarray(end_logits), dtype=np.float32).reshape(P, FREE)
    gt2 = np.concatenate(
        [
            np.asarray(gt_start_positions).astype(np.float32) - np.float32(OFF),
            np.asarray(gt_end_positions).astype(np.float32) - np.float32(OFF),
        ]
    ).reshape(1, 2 * B)
    core_map = {"sl": slf, "el": elf, "cid": cidf, "gt2": gt2, **consts}
    return [dict(core_map) for _ in range(NCORES)]


def kernel(c_ids, gt_start_positions, gt_end_positions, start_logits, end_logits):
    if "nc" not in _CACHE:
        _CACHE["nc"] = _build_program()
    nc = _CACHE["nc"]
    in_maps = build_in_maps(
        c_ids, gt_start_positions, gt_end_positions, start_logits, end_logits
    )
    res = run_bass_kernel_spmd(nc, in_maps, core_ids=list(range(NCORES)))
    return np.asarray(res.results[0]["loss"], dtype=np.float32).reshape(())


# revision 9
# speedup vs baseline: 1.1724x; 1.1133x over previous
"""Trainium2 Bass kernel for nn_DIoUAnswerSpanLoss.

Algorithm
---------
The reference builds a [B, L, L] score matrix score[b,i,j] = ls[b,i] + le[b,j]
(log-softmaxed logits), masks the lower triangle and pad positions, takes
argmax over (i, j), then computes a DIoU-style loss from the resulting integer
span positions (and the ground-truth spans).

Two exact reductions make this cheap:
  1. log_softmax subtracts a per-row constant, which never changes any
     argmax, so raw logits can be used directly for position finding.
  2. max_i<=j (msl[i] + el[j]) = cummax(msl)[j] + el[j], so the [L, L]
     matrix never needs materializing - a prefix max suffices.

Per row: msl = mask ? sl : -BIG; m = cummax(msl); col = mask ? m + el : -BIG;
j* = first argmax(col); v* = m[j*]; i* = first index with m == v*.

Distribution: the whole computation per core is ~15us of straight-line work,
while an 8-core AllReduce of even 16 bytes costs ~45us of latency on this
fabric. So instead of sharding the batch and reducing, every core receives
the full input and computes the full scalar loss independently - zero
communication - and the answer is read from core 0.

Layout: each of the 32 rows occupies 4 SBUF partitions x 512 elements
(p = 4*row + seg, global index g = 512*p + f - 65536, kept negative so a
zero from a non-matching lane never wins a reduce_min). The prefix max runs
as a segmented tensor_tensor_scan; segment boundaries are stitched with a
second scan over the PE-transposed segment ends (a -1e30 additive "reset" at
each row start keeps rows independent). Per-partition (max, first-argmax)
come from the native MAX8/FIND_INDEX8 pair. Cross-partition moves use fp32
matmuls with one-hot constants (exact: every product is x*1.0 or x*0.0);
the per-row selection then runs in a [32 rows x 4 seg] domain where the row
reductions are plain per-partition reduces and every broadcast is a
per-partition tensor_scalar operand. First-occurrence argmax semantics
(matching jnp.argmax) fall out of reduce_min over (value==max)*(negative
global index). The global-index offset cancels in every DIoU term after a
single re-basing subtract, so ground-truth positions are passed pre-offset.
"""

import numpy as np

import concourse.bass as bass
import concourse.bacc as bacc
import concourse.mybir as mybir
from concourse import tile
from concourse.bass_utils import run_bass_kernel_spmd

B, L = 32, 2048
NCORES = 8
SEG = 4                    # segments (partitions) per row
FREE = L // SEG            # 512 elements per segment
P = B * SEG                # 128 partitions
NEGBIG = -1.0e30
GOFF = float(P * FREE)     # 65536: global index offset
OFF = 4096.0               # offset used for the DIoU position space

F32 = mybir.dt.float32
U32 = mybir.dt.uint32
A = mybir.AluOpType
AX = mybir.AxisListType.X

_CACHE = {}


def _tt(nc, out, a, b, op):
    nc.vector.tensor_tensor(out, a, b, op)


def _build_program():
    nc = bacc.Bacc("TRN2", target_bir_lowering=False, debug=False, num_devices=NCORES)

    sl = nc.dram_tensor("sl", [P, FREE], F32, kind="ExternalInput")
    el = nc.dram_tensor("el", [P, FREE], F32, kind="ExternalInput")
    cid = nc.dram_tensor("cid", [P, FREE], F32, kind="ExternalInput")
    gt32 = nc.dram_tensor("gt32", [B, 2], F32, kind="ExternalInput")
    rb32 = nc.dram_tensor("rb32", [B, 2], F32, kind="ExternalInput")
    id128 = nc.dram_tensor("id128", [P, P], F32, kind="ExternalInput")
    ones1 = nc.dram_tensor("ones1", [1, 1], F32, kind="ExternalInput")
    ones32 = nc.dram_tensor("ones32", [B, 1], F32, kind="ExternalInput")
    emat = nc.dram_tensor("emat", [B, P], F32, kind="ExternalInput")
    ematt = nc.dram_tensor("ematt", [P, B], F32, kind="ExternalInput")
    sel4 = nc.dram_tensor("sel4", [P, SEG], F32, kind="ExternalInput")
    resetv = nc.dram_tensor("resetv", [1, P], F32, kind="ExternalInput")
    iotag = nc.dram_tensor("iotag", [P, FREE], F32, kind="ExternalInput")
    loss = nc.dram_tensor("loss", [1], F32, kind="ExternalOutput")

    with tile.TileContext(nc) as tc:
        with (
            tc.tile_pool(name="sb", bufs=1) as sb,
            tc.tile_pool(name="ps", bufs=4, space="PSUM") as ps,
        ):
            # ---- loads (critical ones first) ----
            sl_s = sb.tile([P, FREE], F32)
            cid_s = sb.tile([P, FREE], F32)
            id128_s = sb.tile([P, P], F32)
            el_s = sb.tile([P, FREE], F32)
            iotag_s = sb.tile([P, FREE], F32)
            resetv_s = sb.tile([1, P], F32)
            ones1_s = sb.tile([1, 1], F32)
            ones32_s = sb.tile([B, 1], F32)
            emat_s = sb.tile([B, P], F32)
            ematt_s = sb.tile([P, B], F32)
            sel4_s = sb.tile([P, SEG], F32)
            gt32_s = sb.tile([B, 2], F32)
            rb32_s = sb.tile([B, 2], F32)
            nc.sync.dma_start(cid_s[:], cid[:])
            nc.sync.dma_start(sl_s[:], sl[:])
            nc.sync.dma_start(id128_s[:], id128[:])
            nc.sync.dma_start(el_s[:], el[:])
            nc.sync.dma_start(iotag_s[:], iotag[:])
            nc.sync.dma_start(resetv_s[:], resetv[:])
            nc.sync.dma_start(ones1_s[:], ones1[:])
            nc.sync.dma_start(ones32_s[:], ones32[:])
            nc.sync.dma_start(emat_s[:], emat[:])
            nc.sync.dma_start(ematt_s[:], ematt[:])
            nc.sync.dma_start(sel4_s[:], sel4[:])
            nc.sync.dma_start(gt32_s[:], gt32[:])
            nc.sync.dma_start(rb32_s[:], rb32[:])

            # ---- masked start logits ----
            penalty = sb.tile([P, FREE], F32)
            nc.vector.tensor_scalar(
                penalty[:], cid_s[:], 0.0, NEGBIG, A.is_equal, A.mult
            )
            msl = sb.tile([P, FREE], F32)
            _tt(nc, msl[:], sl_s[:], penalty[:], A.add)

            # ---- segmented prefix max (within each 512-elem segment) ----
            mseg = sb.tile([P, FREE], F32)
            nc.vector.tensor_tensor_scan(
                mseg[:], msl[:], msl[:], NEGBIG, A.max, A.max
            )

            # ---- stitch segments: exclusive cross-segment prefix max ----
            psA = ps.tile([1, P], F32, tag="ps")
            nc.tensor.matmul(psA[:], mseg[:, FREE - 1 : FREE], id128_s[:])
            inclT = sb.tile([1, P], F32)
            nc.vector.tensor_tensor_scan(
                inclT[:], resetv_s[:], psA[:], NEGBIG, A.add, A.max
            )
            exclT = sb.tile([1, P], F32)
            nc.vector.memset(exclT[0:1, 0:1], NEGBIG)
            _tt(nc, exclT[0:1, 1:P], inclT[0:1, 0 : P - 1], resetv_s[0:1, 1:P], A.add)
            psE = ps.tile([P, 1], F32, tag="ps")
            nc.tensor.matmul(psE[:], exclT[:], ones1_s[:])
            m = sb.tile([P, FREE], F32)
            nc.vector.tensor_scalar(m[:], mseg[:], psE[:], None, A.max)

            # ---- column scores: col = m + el + penalty ----
            mel = sb.tile([P, FREE], F32)
            _tt(nc, mel[:], el_s[:], penalty[:], A.add)
            col = sb.tile([P, FREE], F32)
            _tt(nc, col[:], m[:], mel[:], A.add)

            # ---- per-partition top-1 value + first index of col ----
            top8 = sb.tile([P, 8], F32)
            nc.vector.max(top8[:], col[:])
            pm = top8[:, 0:1]
            idx8 = sb.tile([P, 8], U32)
            nc.vector.max_index(idx8[:], top8[:], col[:])
            idxf = sb.tile([P, 1], F32)
            nc.vector.tensor_copy(idxf[:], idx8[:, 0:1])
            pjt = sb.tile([P, 1], F32)
            pj = pjt[:]
            nc.vector.tensor_scalar(pj, idxf[:], iotag_s[:, 0:1], None, A.add)

            # ---- per-partition m value at that index (exact gather) ----
            candv = sb.tile([P, FREE], F32)
            nc.vector.scalar_tensor_tensor(
                candv[:], iotag_s[:], pj, m[:], A.is_equal, A.mult
            )
            pvt = sb.tile([P, 1], F32)
            pv = pvt[:]
            nc.vector.reduce_sum(pv, candv[:], axis=AX)

            # ---- fold [128,1] vectors into [32 rows x 4 seg] ----
            rhs12 = sb.tile([P, 3 * SEG], F32)
            nc.vector.tensor_scalar(rhs12[:, 0:SEG], sel4_s[:], pm, None, A.mult)
            nc.vector.tensor_scalar(
                rhs12[:, SEG : 2 * SEG], sel4_s[:], pj, None, A.mult
            )
            nc.vector.tensor_scalar(
                rhs12[:, 2 * SEG : 3 * SEG], sel4_s[:], pv, None, A.mult
            )
            psT = ps.tile([B, 3 * SEG], F32, tag="ps")
            nc.tensor.matmul(psT[:], ematt_s[:], rhs12[:])
            sT = sb.tile([B, 3 * SEG], F32)
            nc.vector.tensor_copy(sT[:], psT[:])
            pmr = sT[:, 0:SEG]
            pjr = sT[:, SEG : 2 * SEG]
            pvr = sT[:, 2 * SEG : 3 * SEG]

            # ---- per-row selection, all per-partition now ----
            m4 = sb.tile([B, 1], F32)
            nc.vector.reduce_max(m4[:], pmr, axis=AX)
            c1 = sb.tile([B, SEG], F32)
            nc.vector.tensor_scalar(c1[:], pmr, m4[:], None, A.is_equal)
            cj = sb.tile([B, SEG], F32)
            _tt(nc, cj[:], c1[:], pjr, A.mult)
            pos32 = sb.tile([B, 2], F32)
            j4 = pos32[:, 1:2]
            nc.vector.tensor_reduce(j4, cj[:], axis=AX, op=A.min)
            c2 = sb.tile([B, SEG], F32)
            nc.vector.tensor_scalar(c2[:], cj[:], j4, None, A.is_equal)
            cv = sb.tile([B, SEG], F32)
            _tt(nc, cv[:], c2[:], pvr, A.mult)
            v4 = sb.tile([B, 1], F32)
            nc.vector.reduce_sum(v4[:], cv[:], axis=AX)

            # ---- broadcast v* back per partition, find i* ----
            psVb = ps.tile([P, 1], F32, tag="ps")
            nc.tensor.matmul(psVb[:], emat_s[:], v4[:])
            candi = sb.tile([P, FREE], F32)
            nc.vector.scalar_tensor_tensor(
                candi[:], m[:], psVb[:], iotag_s[:], A.is_equal, A.mult
            )
            pit = sb.tile([P, 1], F32)
            pi = pit[:]
            nc.vector.tensor_reduce(pi, candi[:], axis=AX, op=A.min)
            rhs4 = sb.tile([P, SEG], F32)
            nc.vector.tensor_scalar(rhs4[:], sel4_s[:], pi, None, A.mult)
            psI = ps.tile([B, SEG], F32, tag="ps")
            nc.tensor.matmul(psI[:], ematt_s[:], rhs4[:])
            i4 = pos32[:, 0:1]
            nc.vector.tensor_reduce(i4, psI[:], axis=AX, op=A.min)

            # ---- DIoU from positions, [32 x *] domain ----
            posn = sb.tile([B, 2], F32)     # i*-4096 || j*-4096 per row
            _tt(nc, posn[:], pos32[:], rb32_s[:], A.subtract)
            ct2 = sb.tile([B, 2], F32)      # te = ep-sp || tg = gep-gsp
            _tt(nc, ct2[:, 0:1], posn[:, 1:2], posn[:, 0:1], A.subtract)
            _tt(nc, ct2[:, 1:2], gt32_s[:, 1:2], gt32_s[:, 0:1], A.subtract)
            cdg = sb.tile([B, 2], F32)      # cd || gcd = (x+1)*0.5
            nc.vector.tensor_scalar(cdg[:], ct2[:], 1.0, 0.5, A.add, A.mult)
            mnx = sb.tile([B, 2], F32)      # min(sp,gsp) || min(ep,gep)
            _tt(nc, mnx[:], posn[:], gt32_s[:], A.min)
            mxx = sb.tile([B, 2], F32)      # max(sp,gsp) || max(ep,gep)
            _tt(nc, mxx[:], posn[:], gt32_s[:], A.max)
            dd = sb.tile([B, 2], F32)       # cd-gcd || max_end-min_start
            _tt(nc, dd[:, 0:1], cdg[:, 0:1], cdg[:, 1:2], A.subtract)
            _tt(nc, dd[:, 1:2], mxx[:, 1:2], mnx[:, 0:1], A.subtract)
            sq3 = sb.tile([B, 3], F32)      # d1^2 || d2^2 || te+tg
            _tt(nc, sq3[:, 0:2], dd[:], dd[:], A.mult)
            _tt(nc, sq3[:, 2:3], ct2[:, 0:1], ct2[:, 1:2], A.add)
            dI = sb.tile([1, 1], F32)       # I = min(ep,gep)[0] - max(sp,gsp)[0]
            _tt(nc, dI[:], mnx[0:1, 1:2], mxx[0:1, 0:1], A.subtract)
            psS = ps.tile([1, 3], F32, tag="ps")
            nc.tensor.matmul(psS[:], ones32_s[:], sq3[:])

            # loss = 1 - I/(s3 - 32*I) + s1/s2
            u = sb.tile([1, 1], F32)
            nc.vector.scalar_tensor_tensor(
                u[:], dI[:], -float(B), psS[0:1, 2:3], A.mult, A.add
            )
            ru = sb.tile([1, 1], F32)
            nc.vector.reciprocal(ru[:], u[:])
            iou = sb.tile([1, 1], F32)
            _tt(nc, iou[:], dI[:], ru[:], A.mult)
            r2 = sb.tile([1, 1], F32)
            nc.vector.reciprocal(r2[:], psS[0:1, 1:2])
            cl = sb.tile([1, 1], F32)
            _tt(nc, cl[:], psS[0:1, 0:1], r2[:], A.mult)
            tmp2 = sb.tile([1, 1], F32)
            nc.vector.scalar_tensor_tensor(
                tmp2[:], iou[:], -1.0, cl[:], A.mult, A.add
            )
            lossv = sb.tile([1, 1], F32)
            nc.vector.tensor_scalar(lossv[:], tmp2[:], 1.0, None, A.add)
            nc.sync.dma_start(loss[:], lossv[:])

    nc.compile()
    return nc


def _constants():
    ones1 = np.ones((1, 1), dtype=np.float32)
    ones32 = np.ones((B, 1), dtype=np.float32)
    emat = np.zeros((B, P), dtype=np.float32)
    for k in range(B):
        emat[k, k * SEG : (k + 1) * SEG] = 1.0
    ematt = np.ascontiguousarray(emat.T)
    sel4 = (np.arange(P)[:, None] % SEG == np.arange(SEG)[None, :]).astype(np.float32)
    resetv = np.zeros((1, P), dtype=np.float32)
    resetv[0, ::SEG] = NEGBIG
    id128 = np.eye(P, dtype=np.float32)
    iotag = (
        float(FREE) * np.arange(P, dtype=np.float32)[:, None]
        + np.arange(FREE, dtype=np.float32)[None, :]
        - GOFF
    ).astype(np.float32)
    # global row base in (idx - 65536) space relative to the (idx - 4096) space
    rb32 = np.repeat(
        (2048.0 * np.arange(B, dtype=np.float32) - (GOFF - OFF))[:, None], 2, axis=1
    ).astype(np.float32)
    return {
        "ones1": ones1,
        "ones32": ones32,
        "emat": emat,
        "ematt": ematt,
        "sel4": sel4,
        "resetv": resetv,
        "id128": id128,
        "iotag": iotag,
        "rb32": rb32,
    }


def build_in_maps(c_ids, gt_start_positions, gt_end_positions, start_logits, end_logits):
    consts = _constants()
    cidf = np.ascontiguousarray(np.asarray(c_ids), dtype=np.float32).reshape(P, FREE)
    slf = np.ascontiguousarray(np.asarray(start_logits), dtype=np.float32).reshape(P, FREE)
    elf = np.ascontiguousarray(np.asarray(end_logits), dtype=np.float32).reshape(P, FREE)
    gt32 = np.stack(
        [
            np.asarray(gt_start_positions).astype(np.float32) - np.float32(OFF),
            np.asarray(gt_end_positions).astype(np.float32) - np.float32(OFF),
        ],
        axis=1,
    ).astype(np.float32)
    core_map = {"sl": slf, "el": elf, "cid": cidf, "gt32": gt32, **consts}
    return [dict(core_map) for _ in range(NCORES)]


def kernel(c_ids, gt_start_positions, gt_end_positions, start_logits, end_logits):
    if "nc" not in _CACHE:
        _CACHE["nc"] = _build_program()
    nc = _CACHE["nc"]
    in_maps = build_in_maps(
        c_ids, gt_start_positions, gt_end_positions, start_logits, end_logits
    )
    res = run_bass_kernel_spmd(nc, in_maps, core_ids=list(range(NCORES)))
    return np.asarray(res.results[0]["loss"], dtype=np.float32).reshape(())


# revision 10
# speedup vs baseline: 1.1830x; 1.0091x over previous
"""Trainium2 Bass kernel for nn_DIoUAnswerSpanLoss.

Algorithm
---------
The reference builds a [B, L, L] score matrix score[b,i,j] = ls[b,i] + le[b,j]
(log-softmaxed logits), masks the lower triangle and pad positions, takes
argmax over (i, j), then computes a DIoU-style loss from the resulting integer
span positions (and the ground-truth spans).

Two exact reductions make this cheap:
  1. log_softmax subtracts a per-row constant, which never changes any
     argmax, so raw logits can be used directly for position finding.
  2. max_i<=j (msl[i] + el[j]) = cummax(msl)[j] + el[j], so the [L, L]
     matrix never needs materializing - a prefix max suffices.

Per row: msl = mask ? sl : -BIG; m = cummax(msl); col = mask ? m + el : -BIG;
j* = first argmax(col); v* = m[j*]; i* = first index with m == v*.

Distribution: the whole computation per core is ~15us of straight-line work,
while an 8-core AllReduce of even 16 bytes costs ~45us of latency on this
fabric. So instead of sharding the batch and reducing, every core receives
the full input and computes the full scalar loss independently - zero
communication - and the answer is read from core 0.

Layout: each of the 32 rows occupies 4 SBUF partitions x 512 elements
(p = 4*row + seg, global index g = 512*p + f - 65536, kept negative so a
zero from a non-matching lane never wins a reduce_min). The prefix max runs
as a segmented tensor_tensor_scan; segment boundaries are stitched with a
second scan over the PE-transposed segment ends (a -1e30 additive "reset" at
each row start keeps rows independent). Per-partition (max, first-argmax)
come from the native MAX8/FIND_INDEX8 pair. Cross-partition moves use fp32
matmuls with one-hot constants (exact: every product is x*1.0 or x*0.0);
the per-row selection then runs in a [32 rows x 4 seg] domain where the row
reductions are plain per-partition reduces and every broadcast is a
per-partition tensor_scalar operand. First-occurrence argmax semantics
(matching jnp.argmax) fall out of reduce_min over (value==max)*(negative
global index). The global-index offset cancels in every DIoU term after a
single re-basing subtract, so ground-truth positions are passed pre-offset.
"""

import numpy as np

import concourse.bass as bass
import concourse.bacc as bacc
import concourse.mybir as mybir
from concourse import tile
from concourse.bass_utils import run_bass_kernel_spmd

B, L = 32, 2048
NCORES = 8
SEG = 4                    # segments (partitions) per row
FREE = L // SEG            # 512 elements per segment
P = B * SEG                # 128 partitions
NEGBIG = -1.0e30
GOFF = float(P * FREE)     # 65536: global index offset
OFF = 4096.0               # offset used for the DIoU position space

F32 = mybir.dt.float32
U16 = mybir.dt.uint16
A = mybir.AluOpType
AX = mybir.AxisListType.X

_CACHE = {}


def _tt(nc, out, a, b, op):
    nc.vector.tensor_tensor(out, a, b, op)


def _build_program():
    nc = bacc.Bacc("TRN2", target_bir_lowering=False, debug=False, num_devices=NCORES)

    sl = nc.dram_tensor("sl", [P, FREE], F32, kind="ExternalInput")
    el = nc.dram_tensor("el", [P, FREE], F32, kind="ExternalInput")
    cid = nc.dram_tensor("cid", [P, FREE], mybir.dt.int16, kind="ExternalInput")
    gt32 = nc.dram_tensor("gt32", [B, 2], F32, kind="ExternalInput")
    rb32 = nc.dram_tensor("rb32", [B, 2], F32, kind="ExternalInput")
    id128 = nc.dram_tensor("id128", [P, P], F32, kind="ExternalInput")
    ones1 = nc.dram_tensor("ones1", [1, 1], F32, kind="ExternalInput")
    ones32 = nc.dram_tensor("ones32", [B, 1], F32, kind="ExternalInput")
    emat = nc.dram_tensor("emat", [B, P], F32, kind="ExternalInput")
    ematt = nc.dram_tensor("ematt", [P, B], F32, kind="ExternalInput")
    sel4 = nc.dram_tensor("sel4", [P, SEG], F32, kind="ExternalInput")
    resetv = nc.dram_tensor("resetv", [1, P], F32, kind="ExternalInput")
    iotag = nc.dram_tensor("iotag", [P, FREE], F32, kind="ExternalInput")
    loss = nc.dram_tensor("loss", [1], F32, kind="ExternalOutput")

    with tile.TileContext(nc) as tc:
        with (
            tc.tile_pool(name="sb", bufs=1) as sb,
            tc.tile_pool(name="ps", bufs=4, space="PSUM") as ps,
        ):
            # ---- loads (critical ones first) ----
            sl_s = sb.tile([P, FREE], F32)
            cid_s = sb.tile([P, FREE], mybir.dt.int16)
            id128_s = sb.tile([P, P], F32)
            el_s = sb.tile([P, FREE], F32)
            iotag_s = sb.tile([P, FREE], F32)
            resetv_s = sb.tile([1, P], F32)
            ones1_s = sb.tile([1, 1], F32)
            ones32_s = sb.tile([B, 1], F32)
            emat_s = sb.tile([B, P], F32)
            ematt_s = sb.tile([P, B], F32)
            sel4_s = sb.tile([P, SEG], F32)
            gt32_s = sb.tile([B, 2], F32)
            rb32_s = sb.tile([B, 2], F32)
            nc.sync.dma_start(cid_s[:], cid[:])
            nc.sync.dma_start(sl_s[:], sl[:])
            nc.sync.dma_start(id128_s[:], id128[:])
            nc.sync.dma_start(el_s[:], el[:])
            nc.sync.dma_start(iotag_s[:], iotag[:])
            nc.sync.dma_start(resetv_s[:], resetv[:])
            nc.sync.dma_start(ones1_s[:], ones1[:])
            nc.sync.dma_start(ones32_s[:], ones32[:])
            nc.sync.dma_start(emat_s[:], emat[:])
            nc.sync.dma_start(ematt_s[:], ematt[:])
            nc.sync.dma_start(sel4_s[:], sel4[:])
            nc.sync.dma_start(gt32_s[:], gt32[:])
            nc.sync.dma_start(rb32_s[:], rb32[:])

            # ---- masked start logits ----
            penalty = sb.tile([P, FREE], F32)
            nc.vector.tensor_scalar(
                penalty[:], cid_s[:], 0.0, NEGBIG, A.is_equal, A.mult
            )
            msl = sb.tile([P, FREE], F32)
            _tt(nc, msl[:], sl_s[:], penalty[:], A.add)

            # ---- segmented prefix max (within each 512-elem segment) ----
            mseg = sb.tile([P, FREE], F32)
            nc.vector.tensor_tensor_scan(
                mseg[:], msl[:], msl[:], NEGBIG, A.max, A.max
            )

            # ---- stitch segments: exclusive cross-segment prefix max ----
            psA = ps.tile([1, P], F32, tag="ps")
            nc.tensor.matmul(psA[:], mseg[:, FREE - 1 : FREE], id128_s[:])
            inclT = sb.tile([1, P], F32)
            nc.vector.tensor_tensor_scan(
                inclT[:], resetv_s[:], psA[:], NEGBIG, A.add, A.max
            )
            exclT = sb.tile([1, P], F32)
            nc.vector.memset(exclT[0:1, 0:1], NEGBIG)
            _tt(nc, exclT[0:1, 1:P], inclT[0:1, 0 : P - 1], resetv_s[0:1, 1:P], A.add)
            psE = ps.tile([P, 1], F32, tag="ps")
            nc.tensor.matmul(psE[:], exclT[:], ones1_s[:])
            m = sb.tile([P, FREE], F32)
            nc.vector.tensor_scalar(m[:], mseg[:], psE[:], None, A.max)

            # ---- column scores: col = m + el + penalty ----
            mel = sb.tile([P, FREE], F32)
            _tt(nc, mel[:], el_s[:], penalty[:], A.add)
            col = sb.tile([P, FREE], F32)
            _tt(nc, col[:], m[:], mel[:], A.add)

            # ---- per-partition top-1 value + first index of col ----
            top8 = sb.tile([P, 8], F32)
            nc.vector.max(top8[:], col[:])
            pm = top8[:, 0:1]
            idx8 = sb.tile([P, 8], U16)
            nc.vector.max_index(idx8[:], top8[:], col[:])
            idxf = sb.tile([P, 1], F32)
            nc.vector.tensor_copy(idxf[:], idx8[:, 0:1])
            pjt = sb.tile([P, 1], F32)
            pj = pjt[:]
            nc.vector.tensor_scalar(pj, idxf[:], iotag_s[:, 0:1], None, A.add)

            # ---- per-partition m value at that index (exact gather) ----
            candv = sb.tile([P, FREE], F32)
            nc.vector.scalar_tensor_tensor(
                candv[:], iotag_s[:], pj, m[:], A.is_equal, A.mult
            )
            pvt = sb.tile([P, 1], F32)
            pv = pvt[:]
            nc.vector.reduce_sum(pv, candv[:], axis=AX)

            # ---- fold [128,1] vectors into [32 rows x 4 seg] ----
            rhs12 = sb.tile([P, 3 * SEG], F32)
            nc.vector.tensor_scalar(rhs12[:, 0:SEG], sel4_s[:], pm, None, A.mult)
            nc.vector.tensor_scalar(
                rhs12[:, SEG : 2 * SEG], sel4_s[:], pj, None, A.mult
            )
            nc.vector.tensor_scalar(
                rhs12[:, 2 * SEG : 3 * SEG], sel4_s[:], pv, None, A.mult
            )
            psT = ps.tile([B, 3 * SEG], F32, tag="ps")
            nc.tensor.matmul(psT[:], ematt_s[:], rhs12[:])
            sT = sb.tile([B, 3 * SEG], F32)
            nc.vector.tensor_copy(sT[:], psT[:])
            pmr = sT[:, 0:SEG]
            pjr = sT[:, SEG : 2 * SEG]
            pvr = sT[:, 2 * SEG : 3 * SEG]

            # ---- per-row selection, all per-partition now ----
            m4 = sb.tile([B, 1], F32)
            nc.vector.reduce_max(m4[:], pmr, axis=AX)
            c1 = sb.tile([B, SEG], F32)
            nc.vector.tensor_scalar(c1[:], pmr, m4[:], None, A.is_equal)
            cj = sb.tile([B, SEG], F32)
            _tt(nc, cj[:], c1[:], pjr, A.mult)
            pos32 = sb.tile([B, 2], F32)
            j4 = pos32[:, 1:2]
            nc.vector.tensor_reduce(j4, cj[:], axis=AX, op=A.min)
            c2 = sb.tile([B, SEG], F32)
            nc.vector.tensor_scalar(c2[:], cj[:], j4, None, A.is_equal)
            cv = sb.tile([B, SEG], F32)
            _tt(nc, cv[:], c2[:], pvr, A.mult)
            v4 = sb.tile([B, 1], F32)
            nc.vector.reduce_sum(v4[:], cv[:], axis=AX)

            # ---- broadcast v* back per partition, find i* ----
            psVb = ps.tile([P, 1], F32, tag="ps")
            nc.tensor.matmul(psVb[:], emat_s[:], v4[:])
            candi = sb.tile([P, FREE], F32)
            nc.vector.scalar_tensor_tensor(
                candi[:], m[:], psVb[:], iotag_s[:], A.is_equal, A.mult
            )
            pit = sb.tile([P, 1], F32)
            pi = pit[:]
            nc.vector.tensor_reduce(pi, candi[:], axis=AX, op=A.min)
            rhs4 = sb.tile([P, SEG], F32)
            nc.vector.tensor_scalar(rhs4[:], sel4_s[:], pi, None, A.mult)
            psI = ps.tile([B, SEG], F32, tag="ps")
            nc.tensor.matmul(psI[:], ematt_s[:], rhs4[:])
            i4 = pos32[:, 0:1]
            nc.vector.tensor_reduce(i4, psI[:], axis=AX, op=A.min)

            # ---- DIoU from positions, [32 x *] domain ----
            posn = sb.tile([B, 2], F32)     # i*-4096 || j*-4096 per row
            _tt(nc, posn[:], pos32[:], rb32_s[:], A.subtract)
            ct2 = sb.tile([B, 2], F32)      # te = ep-sp || tg = gep-gsp
            _tt(nc, ct2[:, 0:1], posn[:, 1:2], posn[:, 0:1], A.subtract)
            _tt(nc, ct2[:, 1:2], gt32_s[:, 1:2], gt32_s[:, 0:1], A.subtract)
            cdg = sb.tile([B, 2], F32)      # cd || gcd = (x+1)*0.5
            nc.vector.tensor_scalar(cdg[:], ct2[:], 1.0, 0.5, A.add, A.mult)
            mnx = sb.tile([B, 2], F32)      # min(sp,gsp) || min(ep,gep)
            _tt(nc, mnx[:], posn[:], gt32_s[:], A.min)
            mxx = sb.tile([B, 2], F32)      # max(sp,gsp) || max(ep,gep)
            _tt(nc, mxx[:], posn[:], gt32_s[:], A.max)
            dd = sb.tile([B, 2], F32)       # cd-gcd || max_end-min_start
            _tt(nc, dd[:, 0:1], cdg[:, 0:1], cdg[:, 1:2], A.subtract)
            _tt(nc, dd[:, 1:2], mxx[:, 1:2], mnx[:, 0:1], A.subtract)
            sq3 = sb.tile([B, 3], F32)      # d1^2 || d2^2 || te+tg
            _tt(nc, sq3[:, 0:2], dd[:], dd[:], A.mult)
            _tt(nc, sq3[:, 2:3], ct2[:, 0:1], ct2[:, 1:2], A.add)
            dI = sb.tile([1, 1], F32)       # I = min(ep,gep)[0] - max(sp,gsp)[0]
            _tt(nc, dI[:], mnx[0:1, 1:2], mxx[0:1, 0:1], A.subtract)
            psS = ps.tile([1, 3], F32, tag="ps")
            nc.tensor.matmul(psS[:], ones32_s[:], sq3[:])

            # loss = 1 - I/(s3 - 32*I) + s1/s2
            u = sb.tile([1, 1], F32)
            nc.vector.scalar_tensor_tensor(
                u[:], dI[:], -float(B), psS[0:1, 2:3], A.mult, A.add
            )
            ru = sb.tile([1, 1], F32)
            nc.vector.reciprocal(ru[:], u[:])
            iou = sb.tile([1, 1], F32)
            _tt(nc, iou[:], dI[:], ru[:], A.mult)
            r2 = sb.tile([1, 1], F32)
            nc.vector.reciprocal(r2[:], psS[0:1, 1:2])
            cl = sb.tile([1, 1], F32)
            _tt(nc, cl[:], psS[0:1, 0:1], r2[:], A.mult)
            tmp2 = sb.tile([1, 1], F32)
            nc.vector.scalar_tensor_tensor(
                tmp2[:], iou[:], -1.0, cl[:], A.mult, A.add
            )
            lossv = sb.tile([1, 1], F32)
            nc.vector.tensor_scalar(lossv[:], tmp2[:], 1.0, None, A.add)
            nc.sync.dma_start(loss[:], lossv[:])

    nc.compile()
    return nc


def _constants():
    ones1 = np.ones((1, 1), dtype=np.float32)
    ones32 = np.ones((B, 1), dtype=np.float32)
    emat = np.zeros((B, P), dtype=np.float32)
    for k in range(B):
        emat[k, k * SEG : (k + 1) * SEG] = 1.0
    ematt = np.ascontiguousarray(emat.T)
    sel4 = (np.arange(P)[:, None] % SEG == np.arange(SEG)[None, :]).astype(np.float32)
    resetv = np.zeros((1, P), dtype=np.float32)
    resetv[0, ::SEG] = NEGBIG
    id128 = np.eye(P, dtype=np.float32)
    iotag = (
        float(FREE) * np.arange(P, dtype=np.float32)[:, None]
        + np.arange(FREE, dtype=np.float32)[None, :]
        - GOFF
    ).astype(np.float32)
    # global row base in (idx - 65536) space relative to the (idx - 4096) space
    rb32 = np.repeat(
        (2048.0 * np.arange(B, dtype=np.float32) - (GOFF - OFF))[:, None], 2, axis=1
    ).astype(np.float32)
    return {
        "ones1": ones1,
        "ones32": ones32,
        "emat": emat,
        "ematt": ematt,
        "sel4": sel4,
        "resetv": resetv,
        "id128": id128,
        "iotag": iotag,
        "rb32": rb32,
    }


def build_in_maps(c_ids, gt_start_positions, gt_end_positions, start_logits, end_logits):
    consts = _constants()
    cidf = np.ascontiguousarray(np.asarray(c_ids), dtype=np.int16).reshape(P, FREE)
    slf = np.ascontiguousarray(np.asarray(start_logits), dtype=np.float32).reshape(P, FREE)
    elf = np.ascontiguousarray(np.asarray(end_logits), dtype=np.float32).reshape(P, FREE)
    gt32 = np.stack(
        [
            np.asarray(gt_start_positions).astype(np.float32) - np.float32(OFF),
            np.asarray(gt_end_positions).astype(np.float32) - np.float32(OFF),
        ],
        axis=1,
    ).astype(np.float32)
    core_map = {"sl": slf, "el": elf, "cid": cidf, "gt32": gt32, **consts}
    return [dict(core_map) for _ in range(NCORES)]


def kernel(c_ids, gt_start_positions, gt_end_positions, start_logits, end_logits):
    if "nc" not in _CACHE:
        _CACHE["nc"] = _build_program()
    nc = _CACHE["nc"]
    in_maps = build_in_maps(
        c_ids, gt_start_positions, gt_end_positions, start_logits, end_logits
    )
    res = run_bass_kernel_spmd(nc, in_maps, core_ids=list(range(NCORES)))
    return np.asarray(res.results[0]["loss"], dtype=np.float32).reshape(())
